# revision 1
# baseline (speedup 1.0000x reference)
"""Trainium2 Bass kernel for nn_Encoder (pre-norm transformer block, LN over
sequence axis) distributed over 8 NeuronCores.

Comm-minimal, latency-packed design:
  - x^T (bf16) replicated; LN1 stats computed per-core for its 128 channels,
    shared via per-batch 4KB AllGathers, folded into QKV weights/biases
    (h never materialized).
  - head-sharded attention (2 heads x 2 batches/core); batch-1 QKV matmuls
    are interleaved into batch-0's attention chunk stream (attention is
    exp/ACT-bound, PE has slack).
  - partial attn @ Wo[rows] in token-major layout; per-batch bf16
    ReduceScatter lands the attention delta already token-sharded;
    y = x_tok + delta (bo folded into x_tok on host).
  - batch-0's y-transposes are interleaved into batch-1's attention stream;
    LN2 stats via 16KB AllGather + local sum.
  - FFN token-sharded per batch; batch-0's FFN covers batch-1's
    ReduceScatter/stats chain; W1 half-resident, W2 streamed.
Collectives: 2x AG(4KB) + 2x RS(0.5MB) + 2x AG(16KB).
"""

import numpy as np
import ml_dtypes
from contextlib import ExitStack

from concourse import bacc, bass_utils
import concourse.bass as bass
import concourse.tile as tile
import concourse.mybir as mybir
from concourse.masks import make_identity

FP32 = mybir.dt.float32
BF16 = mybir.dt.bfloat16
AF = mybir.ActivationFunctionType
ALU = mybir.AluOpType
AX = mybir.AxisListType

B, T, C, H, HS = 2, 2048, 1024, 16, 64
NCORE, P = 8, 128
TN = B * T            # 4096 flat tokens
TOK = TN // NCORE     # 512 tokens per core (256 per batch)
TB = TOK // B         # 256 tokens per batch per core
F = 4 * C             # 4096
KK = C // P           # 8 k-tiles over C
M = F // P            # 32 m-blocks over F
MH = M // 2           # resident half of W1
EPS = 1e-5
RG = [list(range(NCORE))]

_cache = {}


def build():
    nc = bacc.Bacc("TRN2", target_bir_lowering=False, debug=False,
                   num_devices=NCORE)

    def EIN(name, shape, dtype):
        return nc.dram_tensor(name, shape, dtype, kind="ExternalInput")

    xt = EIN("xt", [C, TN], BF16)          # x^T full (replicated)
    xmine = EIN("xmine", [P, TN], BF16)    # my 128 channels of x^T
    xtok = EIN("xtok", [TOK, C], FP32)     # my token rows, +bo folded in
    wq = EIN("wq", [P, KK, P], BF16)       # Wq cat(2 heads) tiled [p, kk, m]
    wk = EIN("wk", [P, KK, P], BF16)
    wv = EIN("wv", [P, KK, P], BF16)
    wor = EIN("wor", [P, C], BF16)         # Wo rows for my heads
    w1t = EIN("w1t", [P, M, KK, P], BF16)  # [p(c in kk), m, kk, mcol]
    w2t = EIN("w2t", [P, M, C], BF16)      # [p(f in q), q, n]
    bq2 = EIN("bq2", [P, 1], FP32)
    bk2 = EIN("bk2", [P, 1], FP32)
    bv2 = EIN("bv2", [P, 1], FP32)
    b1t = EIN("b1t", [P, M], FP32)         # [p, m]
    b2r = EIN("b2r", [1, C], BF16)         # b2 row (added via ones-row matmul)
    g1 = EIN("g1", [P, 1], FP32)           # LN1 gamma/beta for my 128 chans
    be1 = EIN("be1", [P, 1], FP32)
    g2f = EIN("g2f", [P, KK], FP32)        # LN2 gamma/beta, all chans (p, kk)
    be2f = EIN("be2f", [P, KK], FP32)
    out = nc.dram_tensor("out", [TOK, C], FP32, kind="ExternalOutput")

    with tile.TileContext(nc) as tc, ExitStack() as ctx:
        const = ctx.enter_context(tc.tile_pool(name="const", bufs=1))
        dram = ctx.enter_context(tc.tile_pool(name="dram", bufs=1, space="DRAM"))
        persist = ctx.enter_context(tc.tile_pool(name="acts", bufs=1))

        # ---------------- DRAM comm tiles ----------------
        ab_in = [dram.tile([P, 2], FP32, name=f"abi{b}") for b in range(B)]
        ab_out = [dram.tile([NCORE * P, 2], FP32, name=f"abo{b}")
                  for b in range(B)]
        rs_in = [dram.tile([T, C], BF16, name=f"rsi{b}") for b in range(B)]
        rs_out = [dram.tile([TB, C], BF16, name=f"rso{b}") for b in range(B)]
        ag_in = [dram.tile([P, 2 * KK], FP32, name=f"agi{b}") for b in range(B)]
        ag_out = [dram.tile([NCORE * P, 2 * KK], FP32, name=f"ago{b}")
                  for b in range(B)]

        with tc.tile_pool(name="attn_acts", bufs=1) as acts, \
             tc.tile_pool(name="ph2l", bufs=4) as ph2l, \
             tc.tile_pool(name="dstg", bufs=3) as dstg:
            qT_sb = acts.tile([P, B, T], BF16)
            kT_sb = acts.tile([P, B, T], BF16)
            vaug = acts.tile([P, B, 2, T // P, 65], BF16)
            attnT = acts.tile([P, TN], BF16)

            p1_ctx = ExitStack()
            p1 = p1_ctx.enter_context(tc.tile_pool(name="p1", bufs=1))
            xm_sb = p1.tile([P, TN], BF16)
            nc.sync.dma_start(xm_sb[:], xmine.ap())
            xt1_sb = p1.tile([P, KK, T], BF16)

            p2_ctx = ExitStack()
            p2 = p2_ctx.enter_context(tc.tile_pool(name="p2", bufs=1))
            xt0_sb = p2.tile([P, KK, T], BF16)
            src_v = xt.ap().rearrange("(kk p) n -> p kk n", p=P)
            for kk in range(2):
                nc.gpsimd.dma_start(xt0_sb[:, kk, :], src_v[:, kk, 0:T])

            ident = const.tile([P, P], FP32)
            make_identity(nc, ident)
            ones1 = const.tile([1, P], FP32)
            nc.vector.memset(ones1[:], 1.0)
            onesc_f = const.tile([P, 1], FP32)
            nc.vector.memset(onesc_f[:], 1.0)
            onesc_b = const.tile([P, 1], BF16)
            nc.vector.memset(onesc_b[:], 1.0)
            ones1b = const.tile([1, P], BF16)
            nc.vector.memset(ones1b[:], 1.0)

            def ldconst(t, shape, dt=FP32):
                s = const.tile(shape, dt, name=t.name + "_sb")
                nc.sync.dma_start(s[:], t.ap())
                return s

            def declconst(t, shape, dt=FP32):
                return const.tile(shape, dt, name=t.name + "_sb")

            g1_sb = ldconst(g1, [P, 1])
            be1_sb = ldconst(be1, [P, 1])
            wq_sb = declconst(wq, [P, KK, P], BF16)
            wk_sb = declconst(wk, [P, KK, P], BF16)
            wv_sb = declconst(wv, [P, KK, P], BF16)
            wor_sb = declconst(wor, [P, C], BF16)
            bq_sb = declconst(bq2, [P, 1])
            bk_sb = declconst(bk2, [P, 1])
            bv_sb = declconst(bv2, [P, 1])
            b1_sb = declconst(b1t, [P, M])
            b2_sb = declconst(b2r, [1, C], BF16)
            g2_sb = declconst(g2f, [P, KK])
            be2_sb = declconst(be2f, [P, KK])

            # long-lived activations
            xtok_sb = persist.tile([P, B * 2, C], FP32)  # my tokens; becomes y
            wqf = [persist.tile([P, KK, P], BF16, name=f"wqf{b}")
                   for b in range(B)]
            wkf = [persist.tile([P, KK, P], BF16, name=f"wkf{b}")
                   for b in range(B)]
            wvf = [persist.tile([P, KK, P], BF16, name=f"wvf{b}")
                   for b in range(B)]
            bqf = [persist.tile([P, 1], FP32, name=f"bqf{b}") for b in range(B)]
            bkf = [persist.tile([P, 1], FP32, name=f"bkf{b}") for b in range(B)]
            cvec = [persist.tile([P, 1], FP32, name=f"cvec{b}")
                    for b in range(B)]
            ab_sb = [persist.tile([P, KK, 2], FP32, name=f"absb{b}")
                     for b in range(B)]
            bb_sb = persist.tile([P, KK, 2], BF16)

            def ln_stats_ab(pool, xsrc, b):
                """A,B coeffs for my 128 chans of batch b -> ab_loc [P,2]."""
                n = T
                s1 = pool.tile([P, 1], FP32, tag="s1")
                s2 = pool.tile([P, 1], FP32, tag="s2")
                scr = pool.tile([P, n], FP32, tag="scr", bufs=1)
                nc.vector.reduce_sum(s1[:], xsrc, axis=AX.X)
                nc.vector.scalar_tensor_tensor(
                    out=scr[:], in0=xsrc, scalar=1.0, in1=xsrc,
                    op0=ALU.mult, op1=ALU.mult, accum_out=s2[:])
                mean = pool.tile([P, 1], FP32, tag="mean")
                nc.vector.tensor_scalar_mul(mean[:], s1[:], 1.0 / n)
                ss = pool.tile([P, 1], FP32, tag="ss")
                nc.vector.tensor_mul(ss[:], s1[:], s1[:])
                var = pool.tile([P, 1], FP32, tag="var")
                nc.vector.scalar_tensor_tensor(
                    out=var[:], in0=ss[:], scalar=-1.0 / n, in1=s2[:],
                    op0=ALU.mult, op1=ALU.add)
                nc.vector.tensor_scalar_mul(var[:], var[:], 1.0 / (n - 1))
                den = pool.tile([P, 1], FP32, tag="den")
                nc.scalar.sqrt(den[:], var[:])
                nc.vector.tensor_scalar_add(den[:], den[:], EPS)
                rden = pool.tile([P, 1], FP32, tag="rden")
                nc.vector.reciprocal(rden[:], den[:])
                abl = pool.tile([P, 2], FP32, tag="abl")
                nc.vector.tensor_mul(abl[:, 0:1], g1_sb[:], rden[:])
                mA = pool.tile([P, 1], FP32, tag="mA")
                nc.vector.tensor_scalar_mul(mA[:], mean[:], abl[:, 0:1])
                nc.vector.tensor_sub(abl[:, 1:2], be1_sb[:], mA[:])
                nc.scalar.dma_start(ab_in[b][:], abl[:])
                if b == 0:
                    # bulk x^T loads queued AFTER the tiny stats DMA so the
                    # AllGather isn't stuck behind them on the DMA engines
                    for kk in range(2, KK):
                        nc.sync.dma_start(xt0_sb[:, kk, :], src_v[:, kk, 0:T])
                nc.gpsimd.collective_compute(
                    "AllGather", ALU.bypass, replica_groups=RG,
                    ins=[ab_in[b].opt()], outs=[ab_out[b].opt()])
                nc.sync.dma_start(
                    ab_sb[b][:],
                    ab_out[b].rearrange("(kk p) s -> p kk s", p=P))
                nc.vector.tensor_copy(bb_sb[:, :, b], ab_sb[b][:, :, 1])

            def fold(b, foldp):
                for wbase, wf in ((wq_sb, wqf), (wk_sb, wkf), (wv_sb, wvf)):
                    for kk in range(KK):
                        nc.vector.tensor_scalar_mul(
                            wf[b][:, kk, :], wbase[:, kk, :],
                            ab_sb[b][:, kk, 0:1])
                for wbase, bias, dst in ((wq_sb, bq_sb, bqf),
                                         (wk_sb, bk_sb, bkf),
                                         (wv_sb, bv_sb, cvec)):
                    ps = foldp.tile([P, 1], FP32, tag="bf")
                    for kk in range(KK):
                        nc.tensor.matmul(
                            ps[:], lhsT=wbase[:, kk, :],
                            rhs=bb_sb[:, kk, b:b + 1],
                            start=(kk == 0), stop=(kk == KK - 1))
                    nc.vector.tensor_add(dst[b][:], ps[:], bias[:])

            def qkv_items(b, xt_src, pool):
                """List of closures, each one PSUM group of batch-b QKV."""
                items = []
                for wf, bias, dst in ((wqf, bqf, qT_sb), (wkf, bkf, kT_sb)):
                    for j in range(T // 512):
                        def fq(wf=wf, bias=bias, dst=dst, j=j):
                            ps = pool.tile([P, 512], FP32, tag="qk", name="qkps")
                            for kk in range(KK):
                                nc.tensor.matmul(
                                    ps[:], lhsT=wf[b][:, kk, :],
                                    rhs=xt_src[:, kk, j * 512:(j + 1) * 512],
                                    start=(kk == 0), stop=(kk == KK - 1))
                            nc.vector.tensor_scalar_add(
                                dst[:, b, j * 512:(j + 1) * 512], ps[:],
                                bias[b][:])
                        items.append(fq)
                for tt in range(T // P):
                    def fv(tt=tt):
                        vps_f = pool.tile([P, 512], FP32, tag="qk", name="vps")
                        vps = vps_f[:, 0:P]
                        for kk in range(KK):
                            nc.tensor.matmul(
                                vps,
                                lhsT=xt_src[:, kk, tt * P:(tt + 1) * P],
                                rhs=wvf[b][:, kk, :],
                                start=(kk == 0), stop=(kk == KK - 1))
                        for hd in range(2):
                            nc.vector.tensor_copy(
                                vaug[:, b, hd, tt, 0:64],
                                vps[:, hd * 64:(hd + 1) * 64])
                    items.append(fv)
                return items

            def attention(b, sp, attp, fill_hd, wide=False):
                for hd in range(2):
                    fill = fill_hd[hd] or []
                    att_h = [attp.tile([65, T // 2], FP32, tag="att",
                                       name=f"att{b}{hd}{jh}")
                             for jh in range(2)]
                    for k in range(T // P):
                        p_tiles = []
                        if wide:
                            for jj in range(2):
                                s_ps = sp.tile([P, 1024], FP32, tag="s")
                                for jq in range(2):
                                    nc.tensor.matmul(
                                        s_ps[:, jq * 512:(jq + 1) * 512],
                                        lhsT=kT_sb[hd * 64:(hd + 1) * 64,
                                                   b, k * P:(k + 1) * P],
                                        rhs=qT_sb[hd * 64:(hd + 1) * 64, b,
                                                  jj * 1024 + jq * 512:
                                                  jj * 1024 + (jq + 1) * 512],
                                        start=True, stop=True)
                                p_sb = ph2l.tile([P, 1024], BF16, tag="pw",
                                                 bufs=2, name=f"pw{jj}")
                                nc.scalar.activation(p_sb[:], s_ps[:], AF.Exp,
                                                     scale=float(HS) ** -0.5)
                                p_tiles.append(p_sb)
                            for jj in range(2):
                                for jq in range(2):
                                    nc.tensor.matmul(
                                        att_h[jj][:, jq * 512:(jq + 1) * 512],
                                        lhsT=vaug[:, b, hd, k, :],
                                        rhs=p_tiles[jj][:, jq * 512:
                                                        (jq + 1) * 512],
                                        start=(k == 0),
                                        stop=(k == T // P - 1))
                        else:
                            for j in range(T // 512):
                                s_ps = sp.tile([P, 512], FP32, tag="s")
                                nc.tensor.matmul(
                                    s_ps[:],
                                    lhsT=kT_sb[hd * 64:(hd + 1) * 64, b,
                                               k * P:(k + 1) * P],
                                    rhs=qT_sb[hd * 64:(hd + 1) * 64, b,
                                              j * 512:(j + 1) * 512],
                                    start=True, stop=True)
                                p_sb = ph2l.tile([P, 512], BF16, tag="p",
                                                 bufs=3, name=f"p{j}")
                                nc.scalar.activation(p_sb[:], s_ps[:], AF.Exp,
                                                     scale=float(HS) ** -0.5)
                                p_tiles.append(p_sb)
                            for j in range(T // 512):
                                nc.tensor.matmul(
                                    att_h[j // 2][:, (j % 2) * 512:
                                                  (j % 2 + 1) * 512],
                                    lhsT=vaug[:, b, hd, k, :],
                                    rhs=p_tiles[j][:],
                                    start=(k == 0), stop=(k == T // P - 1))
                        if fill:
                            it = fill.pop(0)
                            if it is not None:
                                it()
                    for jh in range(2):
                        rden = ph2l.tile([1, T // 2], FP32, tag="rden", bufs=1)
                        nc.vector.reciprocal(rden[:], att_h[jh][64:65, :])
                        for jq in range(2):
                            rdf = sp.tile([P, 512], FP32, tag="s", name="rdps")
                            nc.tensor.matmul(
                                rdf[0:64, :], lhsT=ones1[:, 0:64],
                                rhs=rden[:, jq * 512:(jq + 1) * 512],
                                start=True, stop=True)
                            rd_sb = ph2l.tile([64, 512], FP32, tag="rd_sb",
                                              bufs=1)
                            nc.vector.tensor_copy(rd_sb[:], rdf[0:64, :])
                            nc.vector.tensor_mul(
                                attnT[hd * 64:(hd + 1) * 64,
                                      b * T + jh * 1024 + jq * 512:
                                      b * T + jh * 1024 + (jq + 1) * 512],
                                att_h[jh][0:64, jq * 512:(jq + 1) * 512],
                                rd_sb[:])
                    while fill:
                        it = fill.pop(0)
                        if it is not None:
                            it()
                for cq in range(4):
                    nc.vector.tensor_scalar_add(
                        attnT[:, b * T + cq * 512:b * T + (cq + 1) * 512],
                        attnT[:, b * T + cq * 512:b * T + (cq + 1) * 512],
                        cvec[b][:])

            def delta_rs(b, sp):
                for tc_i in range(T // P):
                    d_sb = dstg.tile([P, C], BF16, tag="dsb", bufs=3)
                    for nh in range(2):
                        dps = sp.tile([P, 512], FP32, tag="s", name="dps")
                        nc.tensor.matmul(
                            dps[:],
                            lhsT=attnT[:, b * T + tc_i * P:
                                       b * T + (tc_i + 1) * P],
                            rhs=wor_sb[:, nh * 512:(nh + 1) * 512],
                            start=True, stop=True)
                        sl = d_sb[:, nh * 512:(nh + 1) * 512]
                        if (2 * tc_i + nh) % 2 == 0:
                            nc.vector.tensor_copy(sl, dps[:])
                        else:
                            nc.scalar.activation(sl, dps[:], AF.Copy,
                                                 scale=1.0)
                    nc.sync.dma_start(
                        rs_in[b][tc_i * P:(tc_i + 1) * P, :], d_sb[:])
                nc.gpsimd.collective_compute(
                    "ReduceScatter", ALU.add, replica_groups=RG,
                    ins=[rs_in[b].opt()], outs=[rs_out[b].opt()])

            # ================= phase A: stats, folds, QKV(0) ===============
            with tc.tile_pool(name="stats", bufs=2) as stats, \
                 tc.tile_pool(name="foldp", bufs=2, space="PSUM") as foldp, \
                 tc.tile_pool(name="qkp", bufs=4, space="PSUM") as qkp:
                ln_stats_ab(stats, xm_sb[:, 0:T], 0)
                # weight consts + batch-1 x^T only after the tiny stats DMA
                for wsb, wt in ((wq_sb, wq), (wk_sb, wk), (wv_sb, wv)):
                    nc.sync.dma_start(wsb[:], wt.ap())
                nc.sync.dma_start(bq_sb[:], bq2.ap())
                nc.sync.dma_start(bk_sb[:], bk2.ap())
                nc.sync.dma_start(bv_sb[:], bv2.ap())
                fold(0, foldp)
                ln_stats_ab(stats, xm_sb[:, T:TN], 1)
                for kk in range(KK):
                    nc.gpsimd.dma_start(xt1_sb[:, kk, :], src_v[:, kk, T:TN])
                nc.sync.dma_start(wor_sb[:], wor.ap())
                nc.sync.dma_start(b1_sb[:], b1t.ap())
                nc.sync.dma_start(b2_sb[:], b2r.ap())
                nc.sync.dma_start(g2_sb[:], g2f.ap())
                nc.sync.dma_start(be2_sb[:], be2f.ap())
                fold(1, foldp)
                for it in qkv_items(0, xt0_sb, qkp):
                    it()
            p2_ctx.close()    # free xt0

            # ====== phase B: attention(0) + QKV(1) fill + delta/RS(0) ======
            with tc.tile_pool(name="sp0", bufs=3, space="PSUM") as sp0, \
                 tc.tile_pool(name="qk1", bufs=1, space="PSUM") as qk1, \
                 tc.tile_pool(name="attp0", bufs=2, space="PSUM") as attp0:
                nc.vector.memset(vaug[:, :, :, :, 64], 1.0)
                fill_b = qkv_items(1, xt1_sb, qk1)
                attention(0, sp0, attp0, [fill_b[0:16], fill_b[16:]])
                delta_rs(0, sp0)
            p1_ctx.close()    # free xm + xt1

            # late pools reuse that SBUF
            late_ctx = ExitStack()
            w1res = late_ctx.enter_context(tc.tile_pool(name="w1res", bufs=1))
            tailp = late_ctx.enter_context(tc.tile_pool(name="tail", bufs=1))
            w1a = w1res.tile([P, MH, KK, P], BF16)
            nc.sync.dma_start(w1a[:, 0:MH // 2, :, :],
                              w1t.ap()[:, 0:MH // 2, :, :])
            nc.gpsimd.dma_start(w1a[:, MH // 2:MH, :, :],
                                w1t.ap()[:, MH // 2:MH, :, :])
            nc.sync.dma_start(
                xtok_sb[:], xtok.ap().rearrange("(tc p) c -> p tc c", p=P))
            yT = tailp.tile([P, KK, TOK], FP32)
            h2T = tailp.tile([P, KK, TOK], BF16)
            uT = tailp.tile([P, M, TOK], BF16)

            with tc.tile_pool(name="ph3l", bufs=1) as ph3l, \
                 tc.tile_pool(name="st2", bufs=2) as st2, \
                 tc.tile_pool(name="ffnl", bufs=3) as ffnl, \
                 tc.tile_pool(name="ffno", bufs=2) as ffno:

                def ph3_prep_items(b, stpool):
                    """y = x + delta; per-channel (sum, sumsq) partials via
                    ones-column matmuls on token-major y (PE partition
                    reduction) -> AllGather.  Keeps the stats collective off
                    the transpose path."""
                    y2 = [st2.tile([P, C], BF16, tag=f"y2{j}", bufs=1,
                                   name=f"y2_{b}{j}") for j in range(2)]

                    def f1():
                        dtok = ph3l.tile([P, 2, C], BF16, tag="dtok")
                        nc.gpsimd.dma_start(
                            dtok[:],
                            rs_out[b].rearrange("(j p) c -> p j c", p=P))
                        for j in range(2):
                            nc.gpsimd.tensor_add(
                                xtok_sb[:, b * 2 + j, :],
                                xtok_sb[:, b * 2 + j, :], dtok[:, j, :])
                            nc.vector.tensor_mul(
                                y2[j][:], xtok_sb[:, b * 2 + j, :],
                                xtok_sb[:, b * 2 + j, :])

                    def f2():
                        stps_f = stpool.tile([P, 512], FP32, tag="s",
                                             name="stps")
                        stps = stps_f[:, 0:4 * KK]
                        for cc in range(KK):
                            for j in range(2):
                                nc.tensor.matmul(
                                    stps[:, 4 * cc + j:4 * cc + j + 1],
                                    lhsT=xtok_sb[:, b * 2 + j,
                                                 cc * P:(cc + 1) * P],
                                    rhs=onesc_f[:], start=True, stop=True)
                                nc.tensor.matmul(
                                    stps[:, 4 * cc + 2 + j:4 * cc + 3 + j],
                                    lhsT=y2[j][:, cc * P:(cc + 1) * P],
                                    rhs=onesc_b[:], start=True, stop=True)
                        sts = st2.tile([P, 4 * KK], FP32, tag="sts")
                        nc.vector.tensor_copy(sts[:], stps[:])
                        st = st2.tile([P, 2 * KK], FP32, tag="st")
                        sv = sts.rearrange("p (k j) -> p k j", j=2)
                        nc.vector.tensor_add(st[:], sv[:, :, 0], sv[:, :, 1])
                        nc.scalar.dma_start(ag_in[b][:], st[:])
                        nc.gpsimd.collective_compute(
                            "AllGather", ALU.bypass, replica_groups=RG,
                            ins=[ag_in[b].opt()], outs=[ag_out[b].opt()])
                    return [f1, f2]

                def ph3_prep(b, stpool):
                    for f in ph3_prep_items(b, stpool):
                        f()

                def ph3_transposes(b, tpp, tag="tp"):
                    for j in range(2):
                        for cc in range(KK):
                            tp_f = tpp.tile([P, 512], FP32, tag=tag, name="tp")
                            tp = tp_f[:, 0:P]
                            nc.tensor.transpose(
                                tp,
                                xtok_sb[:, b * 2 + j, cc * P:(cc + 1) * P],
                                ident[:])
                            nc.vector.tensor_copy(
                                yT[:, cc, b * TB + j * P:
                                   b * TB + (j + 1) * P], tp)

                def ph3_ab2(b):
                    stg = st2.tile([P, NCORE, 2 * KK], FP32, tag="stg")
                    nc.gpsimd.dma_start(
                        stg[:], ag_out[b].rearrange("(r p) s -> p r s", p=P))
                    for step in (4, 2, 1):
                        nc.vector.tensor_add(
                            stg[:, 0:step, :], stg[:, 0:step, :],
                            stg[:, step:2 * step, :])
                    stf = stg[:, 0, :].rearrange("p (k s) -> p k s", s=2)
                    mean2 = st2.tile([P, KK], FP32, tag="mean2")
                    nc.vector.tensor_scalar_mul(mean2[:], stf[:, :, 0], 1.0 / T)
                    ss2 = st2.tile([P, KK], FP32, tag="ss2")
                    nc.vector.tensor_mul(ss2[:], stf[:, :, 0], stf[:, :, 0])
                    var2 = st2.tile([P, KK], FP32, tag="var2")
                    nc.vector.scalar_tensor_tensor(
                        out=var2[:], in0=ss2[:], scalar=-1.0 / T,
                        in1=stf[:, :, 1], op0=ALU.mult, op1=ALU.add)
                    nc.vector.tensor_scalar_mul(var2[:], var2[:], 1.0 / (T - 1))
                    den2 = st2.tile([P, KK], FP32, tag="den2")
                    nc.scalar.sqrt(den2[:], var2[:])
                    nc.vector.tensor_scalar_add(den2[:], den2[:], EPS)
                    rden2 = st2.tile([P, KK], FP32, tag="rden2")
                    nc.vector.reciprocal(rden2[:], den2[:])
                    A2 = st2.tile([P, KK], FP32, tag="A2")
                    nc.vector.tensor_mul(A2[:], g2_sb[:], rden2[:])
                    mA2 = st2.tile([P, KK], FP32, tag="mA2")
                    nc.vector.tensor_mul(mA2[:], mean2[:], A2[:])
                    B2 = st2.tile([P, KK], FP32, tag="B2")
                    nc.vector.tensor_sub(B2[:], be2_sb[:], mA2[:])
                    return A2, B2

                def ph3_h2(b, ab2):
                    A2, B2 = ab2
                    for kk in range(KK):
                        nc.vector.tensor_scalar(
                            out=h2T[:, kk, b * TB:(b + 1) * TB],
                            in0=yT[:, kk, b * TB:(b + 1) * TB],
                            scalar1=A2[:, kk:kk + 1], scalar2=B2[:, kk:kk + 1],
                            op0=ALU.mult, op1=ALU.add)

                def ph3_finish(b):
                    ph3_h2(b, ph3_ab2(b))

                def ffn_w1(b, up):
                    for m in range(M):
                        if m < MH:
                            w1_sl = w1a[:, m, :, :]
                        else:
                            w1_t = ffnl.tile([P, KK, P], BF16, tag="w1",
                                             bufs=3)
                            nc.sync.dma_start(w1_t[:], w1t.ap()[:, m, :, :])
                            w1_sl = w1_t[:]
                        ups = up.tile([P, TB], FP32, tag="u")
                        for kk in range(KK):
                            nc.tensor.matmul(
                                ups[:], lhsT=w1_sl[:, kk, :],
                                rhs=h2T[:, kk, b * TB:(b + 1) * TB],
                                start=(kk == 0), stop=(kk == KK - 1))
                        nc.scalar.activation(
                            uT[:, m, b * TB:(b + 1) * TB], ups[:], AF.Relu,
                            bias=b1_sb[:, m:m + 1], scale=1.0)

                def ffn_w2(b, zp, mid_cb=None):
                    zt = [zp.tile([P, C], FP32, tag="z", name=f"z{b}{j}")
                          for j in range(2)]
                    for q in range(M):
                        if q == M // 2 and mid_cb is not None:
                            mid_cb()
                        w2_sl = ffnl.tile([P, C], BF16, tag="w2", bufs=3)
                        nc.sync.dma_start(w2_sl[:], w2t.ap()[:, q, :])
                        for j in range(2):
                            for nh in range(2):
                                nc.tensor.matmul(
                                    zt[j][:, nh * 512:(nh + 1) * 512],
                                    lhsT=uT[:, q, b * TB + j * P:
                                            b * TB + (j + 1) * P],
                                    rhs=w2_sl[:, nh * 512:(nh + 1) * 512],
                                    start=(q == 0), stop=False)
                    for j in range(2):
                        tc_i = b * 2 + j
                        for nh in range(2):
                            nc.tensor.matmul(
                                zt[j][:, nh * 512:(nh + 1) * 512],
                                lhsT=ones1b[:, 0:P],
                                rhs=b2_sb[:, nh * 512:(nh + 1) * 512],
                                start=False, stop=True)
                        o_sb = ffno.tile([P, C], FP32, tag="o", bufs=1)
                        nc.vector.tensor_add(o_sb[:], zt[j][:],
                                             xtok_sb[:, tc_i, :])
                        nc.sync.dma_start(
                            out.ap()[tc_i * P:(tc_i + 1) * P, :], o_sb[:])

                # ========== phase C: attention(1) + ph3(0) fill ==========
                with tc.tile_pool(name="sp1", bufs=2, space="PSUM") as sp1, \
                     tc.tile_pool(name="attp1", bufs=2, space="PSUM") as attp1:
                    pits = ph3_prep_items(0, sp1)
                    ab2_box = []
                    fill_t = [None] * 4 + pits + [None] * 7 + \
                        [lambda: ab2_box.append(ph3_ab2(0))]
                    attention(1, sp1, attp1, [None, fill_t], wide=True)
                    delta_rs(1, sp1)
                    ph3_transposes(0, sp1, tag="s")
                    ph3_h2(0, ab2_box[0])

                # ================= phase D: FFN + ph3(1) =================
                with tc.tile_pool(name="tpp2", bufs=1, space="PSUM") as tpp2, \
                     tc.tile_pool(name="ffp", bufs=2, space="PSUM") as up:
                    ffn_w1(0, up)
                with tc.tile_pool(name="tpp2b", bufs=2, space="PSUM") as tpp2b, \
                     tc.tile_pool(name="zp0", bufs=2, space="PSUM") as zp0:
                    def mid():
                        ph3_prep(1, tpp2b)
                        ph3_transposes(1, tpp2b)
                    ffn_w2(0, zp0, mid_cb=mid)
                    ph3_finish(1)
                with tc.tile_pool(name="ffp1", bufs=2, space="PSUM") as up1:
                    ffn_w1(1, up1)
                with tc.tile_pool(name="zp1", bufs=2, space="PSUM") as zp1:
                    ffn_w2(1, zp1)
            late_ctx.close()

    nc.compile()
    return nc


def prep_inputs(x, Wq, bq, Wk, bk, Wv, bv, Wo, bo, W1, b1, W2, b2,
                gamma1, beta1, gamma2, beta2):
    bf = ml_dtypes.bfloat16
    xf = np.asarray(x, np.float32).reshape(TN, C)
    xt_full = np.ascontiguousarray(xf.T).astype(bf)          # [C, TN]
    w1_full = np.ascontiguousarray(
        np.asarray(W1, np.float32).reshape(KK, P, M, P)
        .transpose(1, 2, 0, 3)).astype(bf)                   # [P, M, KK, P]
    w2_full = np.ascontiguousarray(
        np.asarray(W2, np.float32).reshape(M, P, C)
        .transpose(1, 0, 2)).astype(bf)                      # [P, M, C]
    b1_t = np.ascontiguousarray(b1.reshape(M, P).T).astype(np.float32)
    g2t = np.ascontiguousarray(gamma2.reshape(KK, P).T).astype(np.float32)
    be2t = np.ascontiguousarray(beta2.reshape(KK, P).T).astype(np.float32)

    in_maps = []
    for i in range(NCORE):
        ci = slice(P * i, P * (i + 1))
        hA, hB = 2 * i, 2 * i + 1

        def tile_km(wcat):  # [C, 128] -> [p, kk, m]
            return np.ascontiguousarray(
                wcat.reshape(KK, P, P).transpose(1, 0, 2)).astype(bf)

        wq_cat = np.concatenate([Wq[hA], Wq[hB]], axis=1)
        wk_cat = np.concatenate([Wk[hA], Wk[hB]], axis=1)
        wv_cat = np.concatenate([Wv[hA], Wv[hB]], axis=1)
        xtok_i = np.concatenate(
            [xf[i * TB:(i + 1) * TB], xf[T + i * TB:T + (i + 1) * TB]],
            axis=0) + np.asarray(bo, np.float32)[None, :]
        in_maps.append({
            "xt": xt_full,
            "xmine": np.ascontiguousarray(xt_full[ci]),
            "xtok": np.ascontiguousarray(xtok_i.astype(np.float32)),
            "wq": tile_km(wq_cat),
            "wk": tile_km(wk_cat),
            "wv": tile_km(wv_cat),
            "wor": np.ascontiguousarray(Wo[ci]).astype(bf),
            "w1t": w1_full,
            "w2t": w2_full,
            "bq2": np.concatenate([bq[hA], bq[hB]])[:, None].astype(np.float32),
            "bk2": np.concatenate([bk[hA], bk[hB]])[:, None].astype(np.float32),
            "bv2": np.concatenate([bv[hA], bv[hB]])[:, None].astype(np.float32),
            "b1t": b1_t,
            "b2r": b2[None, :].astype(np.float32).astype(bf),
            "g1": gamma1[ci][:, None].astype(np.float32),
            "be1": beta1[ci][:, None].astype(np.float32),
            "g2f": g2t,
            "be2f": be2t,
        })
    return in_maps


def kernel(**inputs):
    inputs = {k: np.asarray(v) for k, v in inputs.items()}
    if "nc" not in _cache:
        _cache["nc"] = build()
    nc = _cache["nc"]
    in_maps = prep_inputs(**inputs)
    res = bass_utils.run_bass_kernel_spmd(nc, in_maps, core_ids=list(range(NCORE)))
    outf = np.zeros((TN, C), np.float32)
    for i in range(NCORE):
        o = res.results[i]["out"]
        outf[i * TB:(i + 1) * TB] = o[0:TB]
        outf[T + i * TB:T + (i + 1) * TB] = o[TB:TOK]
    return outf.reshape(B, T, C).astype(np.float32)

